# revision 5
# baseline (speedup 1.0000x reference)
"""CRF NLL kernel for Trainium2 (Bass/Tile), SPMD over 8 NeuronCores.

Math: the forward-algorithm logsumexp DP is computed in the exp domain:
    alpha_{t}[j] = log( sum_i exp(alpha_{t-1}[i]) * ET[i,j] ) + em[t,j]
with ET = exp(transitions) (structural -1e9 entries become exact 0s).
Keeping state as ea = exp(alpha - L) with a running per-batch normalizer
L (renormalized every R steps by SumEA, obtained for free from an
appended all-ones column of ET), each step is a plain matmul
(896x896 ET, batch=4 per core) that runs on the TensorEngine.

Layout: state index on partitions (7 tiles of 128, padded 771->896),
batch on the free dim. Both matmul operands and results stay in this
layout, so the scan needs no per-step transposes.

Gold-path score: indirect-DMA gathers of em[b,t,tag] and
trans[prev,cur] using host-precomputed int32 offset tables (index
arithmetic only), reduced with ones-matmuls on device.
"""

import sys

sys.path.insert(0, "/opt/trn_rl_repo")

import numpy as np

import concourse.bass as bass
import concourse.bacc as bacc
import concourse.mybir as mybir
from concourse import tile
from concourse.bass_utils import run_bass_kernel_spmd
from concourse.masks import make_identity

F32 = mybir.dt.float32
BF16 = mybir.dt.bfloat16
I32 = mybir.dt.int32

B, T, S = 32, 128, 771
SP = 896            # padded state count (7 * 128)
KT = SP // 128      # 7 state tiles
NCORES = 8
BL = B // NCORES    # 4 batches per core
BOS, EOS = 0, 1
ONES_COL = 800      # partition 32 of state-tile 6: exp-domain ones column -> SumEA row
PSUM_ROW = 32       # partition of SumEA within tile 6
RENORM = 6          # renormalize every RENORM scan steps (fp32-range safe)
MINUS_INF = -1.0e9

Exp = mybir.ActivationFunctionType.Exp
Ln = mybir.ActivationFunctionType.Ln
MUL = mybir.AluOpType.mult
ADD = mybir.AluOpType.add
SUB = mybir.AluOpType.subtract


def build_program():
    nc = bacc.Bacc(
        "TRN2",
        target_bir_lowering=False,
        debug=False,
        enable_asserts=False,
        num_devices=NCORES,
    )

    # ---- DRAM I/O (per-core shard) ----
    em_flat = nc.dram_tensor("em", [BL * T * SP, 1], F32, kind="ExternalInput").ap()
    tr_flat = nc.dram_tensor("trans", [SP * SP, 1], F32, kind="ExternalInput").ap()
    offs_d = nc.dram_tensor("offs", [T, 2 * BL], I32, kind="ExternalInput").ap()
    off2_d = nc.dram_tensor("off2", [BL, 1], I32, kind="ExternalInput").ap()
    mask_d = nc.dram_tensor("mask", [BL, T], F32, kind="ExternalInput").ap()
    out_d = nc.dram_tensor("out", [BL, 1], F32, kind="ExternalOutput").ap()

    em3 = em_flat.rearrange("(b t s) one -> b t (s one)", b=BL, t=T, s=SP)
    tr2 = tr_flat.rearrange("(i j) one -> i (j one)", i=SP, j=SP)

    with tile.TileContext(nc) as tc:
        with (
            tc.tile_pool(name="const", bufs=1) as cpool,
            tc.tile_pool(name="big", bufs=1) as bigpool,
            tc.tile_pool(name="stage", bufs=2) as stpool,
            tc.tile_pool(name="work", bufs=2) as wpool,
            tc.tile_pool(name="ea", bufs=2) as eapool,
            tc.tile_pool(name="ps", bufs=1, space="PSUM") as pspool,
        ):
            # ---- constants ----
            ident = cpool.tile([128, 128], F32, tag="ident")
            make_identity(nc, ident[:])
            ones = cpool.tile([128, 128], F32, tag="ones")
            nc.gpsimd.memset(ones[:], 1.0)

            # ---- ET = exp(transitions), bf16, (i-part, j-free) 7 tiles ----
            et = []
            for k in range(KT):
                raw = stpool.tile([128, SP], F32, tag="traw")
                nc.sync.dma_start(raw[:], tr2[128 * k : 128 * (k + 1), :])
                etk = bigpool.tile([128, SP], BF16, tag=f"et{k}")
                nc.scalar.activation(etk[:], raw[:], Exp)
                et.append(etk)

            # ---- etBOS[j] = exp(trans[BOS, j]) as (128,1) per j-tile ----
            etbos = []
            for k in range(KT):
                braw = wpool.tile([128, 1], F32, tag="bosraw")
                nc.sync.dma_start(
                    braw[:], tr2[BOS : BOS + 1, 128 * k : 128 * (k + 1)]
                )
                bk = cpool.tile([128, 1], F32, tag=f"etbos{k}")
                nc.scalar.activation(bk[:], braw[:], Exp)
                etbos.append(bk)

            # ---- expem[j-tile] = exp(em), bf16 (j-part, b*128+t free) ----
            expem = []
            for k in range(KT):
                expem.append(bigpool.tile([128, BL * T], BF16, tag=f"xm{k}", name=f"xm{k}"))
            for b in range(BL):
                raw = stpool.tile([128, SP], F32, tag="emraw")
                nc.sync.dma_start(raw[:], em3[b])
                for k in range(KT):
                    tp = pspool.tile([128, 128], F32, tag=f"ps{k}")
                    nc.tensor.transpose(
                        tp[:], raw[:, 128 * k : 128 * (k + 1)], ident[:]
                    )
                    nc.scalar.activation(
                        expem[k][:, 128 * b : 128 * (b + 1)], tp[:], Exp
                    )

            def em_slice(k, t):
                v = expem[k][:].rearrange("p (b t) -> p b t", b=BL)
                return v[:, :, t]

            # ---- running normalizer L (partition 3 of row-3 tiles) ----
            lacc = cpool.tile([128, BL], F32, tag="lacc")
            nc.vector.memset(lacc[:], 0.0)

            # ---- ea_0 = etBOS * expem[:, t=0] ----
            ea = []
            for k in range(KT):
                e0 = eapool.tile([128, BL], BF16, tag=f"ea{k}")
                nc.vector.tensor_scalar_mul(e0[:], em_slice(k, 0), etbos[k][:])
                ea.append(e0)

            # ---- the scan ----
            j_order = [KT - 1] + list(range(KT - 1))  # SumEA tile first
            for t in range(1, T):
                norm = (t % RENORM) == 0
                psums = {}
                recip = None
                for j in j_order:
                    ps = pspool.tile([128, BL], F32, tag=f"ps{j}")
                    for i in range(KT):
                        nc.tensor.matmul(
                            ps[:],
                            et[i][:, 128 * j : 128 * (j + 1)],
                            ea[i][:],
                            start=(i == 0),
                            stop=(i == KT - 1),
                        )
                    psums[j] = ps
                    if j == KT - 1 and norm:
                        # SumEA lives at partition PSUM_ROW of this psum tile
                        sume = wpool.tile([128, BL], F32, tag="sume")
                        nc.vector.tensor_copy(sume[PSUM_ROW:PSUM_ROW+1, :], ps[PSUM_ROW:PSUM_ROW+1, :])
                        bc = pspool.tile([128, BL], F32, tag="bcast")
                        nc.tensor.matmul(
                            bc[:], ones[PSUM_ROW:PSUM_ROW+1, :], sume[PSUM_ROW:PSUM_ROW+1, :],
                            start=True, stop=True,
                        )
                        recip = wpool.tile([128, BL], F32, tag="recip")
                        nc.vector.reciprocal(recip[:], bc[:])
                        lns = wpool.tile([128, BL], F32, tag="lns")
                        nc.scalar.activation(lns[PSUM_ROW:PSUM_ROW+1, :], sume[PSUM_ROW:PSUM_ROW+1, :], Ln)
                        nc.vector.tensor_tensor(
                            lacc[PSUM_ROW:PSUM_ROW+1, :], lacc[PSUM_ROW:PSUM_ROW+1, :], lns[PSUM_ROW:PSUM_ROW+1, :], op=ADD
                        )
                ea_next = []
                for j in range(KT):
                    ej = eapool.tile([128, BL], BF16, tag=f"ea{j}")
                    if norm:
                        tmp = wpool.tile([128, BL], F32, tag=f"tmp{j}")
                        nc.vector.tensor_tensor(
                            tmp[:], psums[j][:], recip[:], op=MUL
                        )
                        nc.vector.tensor_tensor(
                            ej[:], tmp[:], em_slice(j, t), op=MUL
                        )
                    else:
                        nc.vector.tensor_tensor(
                            ej[:], psums[j][:], em_slice(j, t), op=MUL
                        )
                    ea_next.append(ej)
                ea = ea_next

            # ---- log Z = L + ln( sum_j ea[j] * exp(trans[j, EOS]) ) ----
            fin = pspool.tile([1, BL], F32, tag="bcast")
            for k in range(KT):
                nc.tensor.matmul(
                    fin[:], et[k][:, EOS : EOS + 1], ea[k][:],
                    start=(k == 0), stop=(k == KT - 1),
                )
            lnfin = wpool.tile([1, BL], F32, tag="lnfin")
            nc.scalar.activation(lnfin[:], fin[:], Ln)
            # transpose (1,BL)@p0 + L(1,BL)@p3 -> (BL,1) via two matmuls
            lzt = pspool.tile([BL, 1], F32, tag="bcast")
            nc.tensor.matmul(
                lzt[:], lnfin[:], ones[0:1, 0:1], start=True, stop=False
            )
            nc.tensor.matmul(
                lzt[:], lacc[PSUM_ROW:PSUM_ROW+1, :], ones[PSUM_ROW:PSUM_ROW+1, 0:1], start=False, stop=True
            )

            # ---- gold path score via indirect gathers ----
            offs = cpool.tile([T, 2 * BL], I32, tag="offs")
            nc.sync.dma_start(offs[:], offs_d[:])
            off2 = cpool.tile([BL, 1], I32, tag="off2")
            nc.sync.dma_start(off2[:], off2_d[:])
            g1 = wpool.tile([T, 2 * BL], F32, tag="g1")
            for c in range(BL):
                nc.gpsimd.indirect_dma_start(
                    out=g1[:, c : c + 1],
                    out_offset=None,
                    in_=em_flat[:],
                    in_offset=bass.IndirectOffsetOnAxis(
                        ap=offs[:, c : c + 1], axis=0
                    ),
                )
            for c in range(BL, 2 * BL):
                nc.gpsimd.indirect_dma_start(
                    out=g1[:, c : c + 1],
                    out_offset=None,
                    in_=tr_flat[:],
                    in_offset=bass.IndirectOffsetOnAxis(
                        ap=offs[:, c : c + 1], axis=0
                    ),
                )
            g2 = wpool.tile([BL, 1], F32, tag="g2")
            nc.gpsimd.indirect_dma_start(
                out=g2[:],
                out_offset=None,
                in_=tr_flat[:],
                in_offset=bass.IndirectOffsetOnAxis(ap=off2[:], axis=0),
            )
            maskt = wpool.tile([T, BL], F32, tag="maskt")
            for b in range(BL):
                nc.sync.dma_start(maskt[:, b : b + 1], mask_d[b : b + 1, :])
            g1m = wpool.tile([T, 2 * BL], F32, tag="g1m")
            nc.vector.tensor_tensor(g1m[:, 0:BL], g1[:, 0:BL], maskt[:], op=MUL)
            nc.vector.tensor_tensor(
                g1m[:, BL : 2 * BL], g1[:, BL : 2 * BL], maskt[:], op=MUL
            )
            sc = pspool.tile([BL, 1], F32, tag="ps0")
            nc.tensor.matmul(
                sc[:], g1m[:, 0:BL], ones[:, 0:1], start=True, stop=False
            )
            nc.tensor.matmul(
                sc[:], g1m[:, BL : 2 * BL], ones[:, 0:1], start=False, stop=True
            )
            score = wpool.tile([BL, 1], F32, tag="scoresb")
            nc.vector.tensor_tensor(score[:], sc[:], g2[:], op=ADD)

            # ---- diff = score - logz ; host sums and negates ----
            diff = wpool.tile([BL, 1], F32, tag="diff")
            nc.vector.tensor_tensor(diff[:], score[:], lzt[:], op=SUB)
            nc.sync.dma_start(out_d[:], diff[:])

    nc.compile()
    return nc


_NC = None


def _get_nc():
    global _NC
    if _NC is None:
        _NC = build_program()
    return _NC


def make_in_maps(emissions, tags, mask, transitions):
    emissions = np.asarray(emissions, dtype=np.float32)
    tags = np.asarray(tags).astype(np.int64)
    mask = np.asarray(mask, dtype=np.float32)
    transitions = np.asarray(transitions, dtype=np.float32)

    em_pad = np.full((B, T, SP), MINUS_INF, dtype=np.float32)
    em_pad[:, :, :S] = emissions
    tr_pad = np.full((SP, SP), MINUS_INF, dtype=np.float32)
    tr_pad[:S, :S] = transitions
    tr_pad[:S, ONES_COL] = 0.0  # exp-domain all-ones column

    last_idx = mask.sum(axis=1).astype(np.int64) - 1
    in_maps = []
    for c in range(NCORES):
        bs = slice(c * BL, (c + 1) * BL)
        tg = tags[bs]  # (BL, T)
        offs = np.empty((T, 2 * BL), dtype=np.int32)
        for bl in range(BL):
            bt = np.arange(T, dtype=np.int64)
            offs[:, bl] = (bl * T + bt) * SP + tg[bl]
            prev = np.concatenate(([np.int64(BOS)], tg[bl, :-1]))
            offs[:, BL + bl] = prev * SP + tg[bl]
        off2 = np.empty((BL, 1), dtype=np.int32)
        for bl in range(BL):
            off2[bl, 0] = tg[bl, last_idx[c * BL + bl]] * SP + EOS
        in_maps.append(
            {
                "em": em_pad[bs].reshape(BL * T * SP, 1),
                "trans": tr_pad.reshape(SP * SP, 1),
                "offs": offs,
                "off2": off2,
                "mask": mask[bs],
            }
        )
    return in_maps


def kernel(emissions, tags, mask, transitions, _want_profile=False):
    nc = _get_nc()
    in_maps = make_in_maps(emissions, tags, mask, transitions)
    res = run_bass_kernel_spmd(
        nc,
        in_maps,
        core_ids=list(range(NCORES)),
        trace=_want_profile,
    )
    diffs = np.concatenate([r["out"].reshape(-1) for r in res.results])
    out = np.float32(-np.sum(diffs.astype(np.float64)))
    if _want_profile:
        return np.asarray(out), res
    return np.asarray(out)
